# revision 1
# baseline (speedup 1.0000x reference)
"""BitNet Transformer MLP on 8 Trainium2 NeuronCores — v2.

Math (per reference):
  sw1 = max(mean|W1|, EPS); wq1 = clip(round(W1/sw1), -1, 1)
  sx[t] = max(max_h|x[t,h]|, EPS)/127; xq = round(x/sx)      (ints in [-127,127])
  h = gelu((xq @ wq1.T) * sx * sw1)                           (exact erf gelu)
  sh[t] = max(max_i|h[t,i]|, EPS)/127; hq = round(h/sh)
  out = (hq @ wq2.T) * sh * sw2

Sharding (tensor-parallel over the intermediate dim I), per core:
  - tokens T flattened; core c quantizes its T/8 token slice (transposed
    layout), AllGather -> xqT (bf16, exact), chunk-interleaved by rank
  - core c holds W1 rows [c*I/8:(c+1)*I/8] and W2 cols [same I-slice],
    host-pre-transposed: w1t=[H, I/8], w2t=[I/8, H], xt=[H, T/8]
  - per-tensor weight scales via a 2-float AllReduce of |W| partial sums
  - fc1 computes the h.T slice [I/8, T] locally; W1 is quantized INLINE
    during its fc1 load (no DRAM round-trip of the quantized copy)
  - per-token max|h| partials -> one AllReduce(max) of [T]
  - fc2 (W2 also quantized inline) computes partial out.T [H, T];
    ReduceScatter(add) per token-block -> core c owns out.T rows
    [c*H/8:(c+1)*H/8]; host concatenates and transposes back.

v2 structure changes vs v1:
  - weight |.| partial sums run CONCURRENTLY with the x-quant phase on a
    separate DMA issue queue (vector) so neither stream head-of-line
    blocks the other; x-phase DMAs issue from scalar; everything from
    fc1 on issues from sync.
  - collective order: AG(sx) -> AG(xq) -> AR(wsum): the big AG fires as
    soon as xq is written; the tiny AR overlaps fc1 weight prefetch.
  - W1/W2 are quantized inline into their resident SBUF tiles (f32 load
    + scale/round/clip), removing the w1q/w2q DRAM write+read and the
    serial weight-quantization phase.

All matmuls run in bf16, which is EXACT here: quantized activations are
integers <=127 and weights are ternary, both exactly representable in
bf16; accumulation is fp32 in PSUM. The intermediate h is spilled in fp16.
"""

import numpy as np

import concourse.bass as bass
import concourse.mybir as mybir
import concourse.tile as tile
from concourse import bass_utils, bacc

F32 = mybir.dt.float32
BF16 = mybir.dt.bfloat16
FP16 = mybir.dt.float16
MAGIC = 12582912.0  # 1.5*2^23: (v+MAGIC)-MAGIC == round-to-nearest-even, |v|<2^22
EPS = 1e-5
Alu = mybir.AluOpType
Act = mybir.ActivationFunctionType

B, S, H, I = 4, 2048, 4096, 16384
T = B * S
NCORES = 8


def build_program(T=T, H=H, I=I, ncores=NCORES, nb=512, w1_halves=2, repeat=1, WSTEP=5.5,
                  rep_quant=True, rep_w1abs=True, rep_w2abs=True):
    TS = T // ncores          # token shard (quant phase)
    IS = I // ncores          # I shard per core
    HS = H // ncores          # H shard of the final output per core
    NBLK = T // nb            # token blocks
    KH = H // 128             # contraction tiles for fc1
    KI = IS // 128            # contraction tiles for fc2
    IH = IS // w1_halves      # fc1 weight-resident half size
    MIH = IH // 128           # fc1 m-tiles per half
    MH = H // 128             # fc2 m-tiles
    NT32 = nb // 32           # 32-token groups per block
    CHUNK = 512
    BPC = TS // nb            # token blocks per AG rank-chunk
    assert nb % 128 == 0 and TS % 128 == 0 and IS % 128 == 0 and TS % nb == 0

    nc = bacc.Bacc("TRN2", target_bir_lowering=False, debug=False, num_devices=ncores)

    xt_e = nc.dram_tensor("xt", [H, TS], F32, kind="ExternalInput")
    w1t_e = nc.dram_tensor("w1t", [H, IS], F32, kind="ExternalInput")
    w2t_e = nc.dram_tensor("w2t", [IS, H], F32, kind="ExternalInput")
    out_e = nc.dram_tensor("out_t", [HS, T], F32, kind="ExternalOutput")

    rg = [list(range(ncores))]
    NPAR = min(2, repeat)

    with tile.TileContext(nc) as tc:
        with (
            tc.tile_pool(name="singles", bufs=1) as singles,
            tc.tile_pool(name="work", bufs=3) as work,
            tc.tile_pool(name="wabs", bufs=2) as wabs,
            tc.tile_pool(name="wqw", bufs=3) as wqw,
            tc.tile_pool(name="bigw", bufs=1) as bigw,
            tc.tile_pool(name="xqp", bufs=2) as xqp,
            tc.tile_pool(name="hqp", bufs=1) as hqp,
            tc.tile_pool(name="stage", bufs=2) as stage,
            tc.tile_pool(name="outp", bufs=2) as outp,
            tc.tile_pool(name="psum", bufs=6, space="PSUM") as psum,
            tc.tile_pool(name="psbc", bufs=2, space="PSUM") as psbc,
            tc.tile_pool(name="dram", bufs=1, space="DRAM") as dram,
        ):
            # ------------- DRAM scratch (parity double-buffered across reps) ------
            xq_ag_in = [dram.tile([H, TS], BF16, name=f"xq_ag_in_{p}") for p in range(NPAR)]
            # chunk-interleaved: rank c's tokens live at rows [c*H:(c+1)*H]
            xqT_full = [dram.tile([ncores * H, TS], BF16, name=f"xqT_full_{r}",
                                  addr_space="Shared") for r in range(repeat)]
            sx_ag_in = [dram.tile([TS], F32, name=f"sx_ag_in_{p}") for p in range(NPAR)]
            sx_full = [dram.tile([T], F32, name=f"sx_full_{r}", addr_space="Shared")
                       for r in range(repeat)]
            wsum1_in = [dram.tile([1, 1], F32, name=f"wsum1_in_{p}") for p in range(NPAR)]
            wsum1_out = [dram.tile([1, 1], F32, name=f"wsum1_out_{r}", addr_space="Shared")
                         for r in range(repeat)]
            wsum2_in = [dram.tile([1, 1], F32, name=f"wsum2_in_{p}") for p in range(NPAR)]
            wsum2_out = [dram.tile([1, 1], F32, name=f"wsum2_out_{r}", addr_space="Shared")
                         for r in range(repeat)]
            h_dram = [dram.tile([IS, T], FP16, name=f"h_dram_{p}") for p in range(NPAR)]
            hmax_in = [dram.tile([T], F32, name=f"hmax_in_{p}") for p in range(NPAR)]
            hmax_out = [dram.tile([T], F32, name=f"hmax_out_{r}", addr_space="Shared")
                        for r in range(repeat)]
            RSG = 4                    # token blocks per ReduceScatter
            NRSG = NBLK // RSG
            rs_in = [[dram.tile([H, RSG * nb], FP16, name=f"rs_in_{p}_{j}") for j in range(NRSG)]
                     for p in range(NPAR)]
            rs_out = [[dram.tile([HS, RSG * nb], FP16, name=f"rs_out_{r}_{j}") for j in range(NRSG)]
                      for r in range(repeat)]

            # ---------------- constants ----------------
            ones_row = singles.tile([1, 128], F32, name="ones_row")
            nc.any.memset(ones_row[:], 1.0)
            ones_col = singles.tile([128, 1], F32, name="ones_col")
            nc.any.memset(ones_col[:], 1.0)

            def bcast_row(row_ap, n, tag="bc"):
                """[1, n] SBUF row -> [128, n] tile (PE ones outer product)."""
                ps = psbc.tile([128, nb], F32, tag="psbc")
                nc.tensor.matmul(ps[:, :n], lhsT=ones_row[:], rhs=row_ap, start=True, stop=True)
                t = stage.tile([128, nb], F32, tag=tag)
                nc.vector.tensor_copy(t[:, :n], ps[:, :n])
                return t

            def bcast_scalar(src_ap, name):
                ps_full = psbc.tile([128, nb], F32, tag="psbc")
                ps = ps_full[:, 0:1]
                nc.tensor.matmul(ps, lhsT=ones_row[:], rhs=src_ap, start=True, stop=True)
                t = singles.tile([128, 1], F32, name=name)
                nc.vector.tensor_copy(t[:], ps)
                return t

            def abs_sum_partial(src, rows, cols, tag):
                acc = singles.tile([128, 1], F32, name=f"acc_{tag}")
                first = True
                for it in range(rows // 128):
                    for c0 in range(0, cols, CHUNK):
                        cw = min(CHUNK, cols - c0)
                        wt = wabs.tile([128, CHUNK], F32, tag="wf32")
                        nc.scalar.dma_start(
                            wt[:, :cw], src[it * 128:(it + 1) * 128, c0:c0 + cw]
                        )
                        part = stage.tile([128, 1], F32, tag="wpart")
                        nc.vector.tensor_reduce(part[:], wt[:, :cw], axis=mybir.AxisListType.X,
                                                op=Alu.add, apply_absolute_value=True)
                        if first:
                            nc.vector.tensor_copy(acc[:], part[:])
                            first = False
                        else:
                            nc.vector.tensor_tensor(acc[:], acc[:], part[:], Alu.add)
                return acc

            def wsum_store(acc, dst, tag):
                wsum_sb = singles.tile([1, 1], F32, name=f"wsum_sb_{tag}")
                pss_full = psbc.tile([128, nb], F32, tag="psbc")
                pss = pss_full[0:1, 0:1]
                nc.tensor.matmul(pss, lhsT=acc[:], rhs=ones_col[:], start=True, stop=True)
                nc.vector.tensor_copy(wsum_sb[0:1, 0:1], pss)
                nc.scalar.dma_start(dst[:, :], wsum_sb[:])

            def make_scales(wsum_out_t, tag):
                sw_sb = singles.tile([1, 1], F32, name=f"sw_sb_{tag}")
                nc.gpsimd.dma_start(sw_sb[:], wsum_out_t[:, :])
                nc.vector.tensor_scalar_mul(sw_sb[:], sw_sb[:], 1.0 / (I * H))
                nc.vector.tensor_scalar_max(sw_sb[:], sw_sb[:], EPS)
                rsw_sb = singles.tile([1, 1], F32, name=f"rsw_sb_{tag}")
                nc.vector.reciprocal(rsw_sb[:], sw_sb[:])
                rsw_col = bcast_scalar(rsw_sb[0:1, 0:1], f"rsw_col_{tag}")
                sw_127_col = bcast_scalar(sw_sb[0:1, 0:1], f"sw127_col_{tag}")
                nc.vector.tensor_scalar_mul(sw_127_col[:], sw_127_col[:], 1.0 / 127.0)
                return rsw_col, sw_127_col

            def quant_into(dst_ap, src_ap, cw, rsw_col):
                """dst = clip(round(src * rsw), -1, 1), f32 -> bf16."""
                nc.scalar.mul(src_ap, src_ap, rsw_col[:])
                nc.vector.tensor_scalar(src_ap, src_ap, MAGIC, MAGIC, Alu.add, Alu.subtract)
                nc.vector.tensor_scalar(dst_ap, src_ap, 1.0, -1.0, Alu.min, Alu.max)

            NQC = TS // CHUNK
            hred_acc = [singles.tile([32, NT32], F32, name=f"hred_{j}")
                        for j in range(NBLK)]

            for _rep in range(repeat):
                p = _rep % NPAR
                W = WSTEP * _rep

                # -------- phase Q: sx + quantize, single resident xt pass --------
                do_quant = rep_quant or _rep == 0
                qrep = _rep if rep_quant else 0
                with tc.tile_wait_until(W):
                    for qc in range(NQC if do_quant else 0):
                        c0 = qc * CHUNK
                        xtc = bigw.tile([128, KH, CHUNK], F32, tag="bigw")
                        amax = stage.tile([128, CHUNK], F32, tag="bc")
                        for it in range(KH):
                            nc.scalar.dma_start(
                                xtc[:, it, :], xt_e[it * 128:(it + 1) * 128, c0:c0 + CHUNK]
                            )
                            if it == 0:
                                nc.scalar.activation(amax[:], xtc[:, it, :], Act.Abs)
                            else:
                                xa = work.tile([128, CHUNK], F32, tag="cf32")
                                nc.scalar.activation(xa[:], xtc[:, it, :], Act.Abs)
                                nc.vector.tensor_tensor(amax[:], amax[:], xa[:], Alu.max)
                        ftm = stage.tile([64, CHUNK], F32, tag="foldx")
                        nc.vector.tensor_copy(ftm[0:64], amax[64:128])
                        nc.vector.tensor_tensor(amax[0:64], amax[0:64], ftm[0:64], Alu.max)
                        nc.vector.tensor_copy(ftm[0:32], amax[32:64])
                        nc.vector.tensor_tensor(amax[0:32], amax[0:32], ftm[0:32], Alu.max)
                        amt = stage.tile([32, CHUNK], F32, tag="foldx")
                        nc.vector.transpose(amt[:], amax[0:32, :])
                        xred = stage.tile([32, CHUNK // 32], F32, tag="xred")
                        nc.vector.tensor_reduce(
                            xred[:], amt[:].rearrange("p (c q) -> p c q", q=32),
                            axis=mybir.AxisListType.X, op=Alu.max,
                        )
                        nc.vector.tensor_scalar_max(xred[:], xred[:], EPS)
                        nc.scalar.dma_start(
                            sx_ag_in[p][c0:c0 + CHUNK].rearrange("(c p) -> p c", p=32),
                            xred[:],
                        )
                        # reciprocal row (round-trip through DRAM for layout)
                        rq_row = stage.tile([1, nb], F32, tag="srow")
                        nc.scalar.dma_start(
                            rq_row[:, :CHUNK],
                            sx_ag_in[p][c0:c0 + CHUNK].rearrange("(a f) -> a f", a=1))
                        nc.vector.reciprocal(rq_row[:, :CHUNK], rq_row[:, :CHUNK])
                        nc.vector.tensor_scalar_mul(rq_row[:, :CHUNK], rq_row[:, :CHUNK], 127.0)
                        ps = psbc.tile([128, nb], F32, tag="psbc")
                        nc.tensor.matmul(ps[:, :CHUNK], lhsT=ones_row[:], rhs=rq_row[:, :CHUNK],
                                         start=True, stop=True)
                        rq_bcc = stage.tile([128, nb], F32, tag="bc")
                        nc.vector.tensor_copy(rq_bcc[:, :CHUNK], ps[:, :CHUNK])
                        for it in range(KH):
                            nc.vector.tensor_tensor(xtc[:, it, :], xtc[:, it, :],
                                                    rq_bcc[:, :CHUNK], Alu.mult)
                            xqt = work.tile([128, CHUNK], BF16, tag="cbf")
                            nc.vector.tensor_scalar(xqt[:], xtc[:, it, :], MAGIC, MAGIC,
                                                    Alu.add, Alu.subtract)
                            nc.scalar.dma_start(
                                xq_ag_in[p][it * 128:(it + 1) * 128, c0:c0 + CHUNK], xqt[:])

                    if do_quant:
                        nc.gpsimd.collective_compute(
                            "AllGather", Alu.bypass, replica_groups=rg,
                            ins=[sx_ag_in[p][:].opt()], outs=[sx_full[qrep][:].opt()],
                        )
                        nc.gpsimd.collective_compute(
                            "AllGather", Alu.bypass, replica_groups=rg,
                            ins=[xq_ag_in[p][:].opt()], outs=[xqT_full[qrep][:].opt()],
                        )

                # -------- W1 |.| sums + AR + scales --------
                if rep_w1abs or _rep == 0:
                    with tc.tile_wait_until(W + 0.05):
                        acc1 = abs_sum_partial(w1t_e, H, IS, "w1")
                        wsum_store(acc1, wsum1_in[p], "w1")
                        nc.gpsimd.collective_compute(
                            "AllReduce", Alu.add, replica_groups=rg,
                            ins=[wsum1_in[p][:].opt()], outs=[wsum1_out[_rep][:].opt()],
                        )
                        rsw1_col, sw1_127_col = make_scales(wsum1_out[_rep], "w1")

                # ---------------- fc1 (inline W1 quantization) ----------------
                for half in range(w1_halves):
                    with tc.tile_wait_until(W + 0.1 + half * 1.1):
                        w1qT = bigw.tile([128, KH, IH], BF16, tag="bigw")
                        for k in range(KH):
                            for c0 in range(0, IH, CHUNK):
                                cw = min(CHUNK, IH - c0)
                                wf = wqw.tile([128, CHUNK], F32, tag="wq32")
                                nc.scalar.dma_start(
                                    wf[:, :cw],
                                    w1t_e[k * 128:(k + 1) * 128,
                                          half * IH + c0: half * IH + c0 + cw],
                                )
                                quant_into(w1qT[:, k, c0:c0 + cw], wf[:, :cw], cw, rsw1_col)
                    for blk in range(NBLK):
                        tc.tile_set_cur_wait(W + 0.35 + half * 1.0)
                        crk = blk // BPC           # AG rank chunk
                        coff = (blk % BPC) * nb    # token offset within chunk
                        xq_sb = xqp.tile([128, KH, nb], BF16, tag="xq")
                        for k in range(KH):
                            nc.sync.dma_start(
                                xq_sb[:, k, :],
                                xqT_full[qrep][crk * H + k * 128: crk * H + (k + 1) * 128,
                                            coff:coff + nb],
                            )
                        s_row = stage.tile([1, nb], F32, tag="srow")
                        nc.sync.dma_start(
                            s_row[:],
                            sx_full[qrep][blk * nb:(blk + 1) * nb].rearrange("(a f) -> a f", a=1),
                        )
                        m1_t = bcast_row(s_row[:], nb)
                        nc.vector.tensor_scalar(m1_t[:], m1_t[:], sw1_127_col[:], None, Alu.mult)

                        gmax = stage.tile([128, nb], FP16, tag="gmax")
                        for m in range(MIH):
                            ps = psum.tile([128, nb], F32, tag="ps1")
                            for k in range(KH):
                                nc.tensor.matmul(
                                    ps[:], lhsT=w1qT[:, k, m * 128:(m + 1) * 128],
                                    rhs=xq_sb[:, k, :],
                                    start=(k == 0), stop=(k == KH - 1),
                                )
                            g = work.tile([128, nb], F32, tag="g")
                            nc.vector.tensor_tensor(g[:], ps[:], m1_t[:], Alu.mult)
                            gq = work.tile([128, nb], FP16, tag="gq")
                            nc.scalar.activation(gq[:], g[:], Act.Gelu)
                            nc.sync.dma_start(
                                h_dram[p][half * IH + m * 128: half * IH + (m + 1) * 128,
                                          blk * nb:(blk + 1) * nb],
                                gq[:],
                            )
                            gabs = work.tile([128, nb], FP16, tag="tmph")
                            nc.scalar.activation(gabs[:], gq[:], Act.Abs)
                            if m == 0:
                                nc.vector.tensor_copy(gmax[:], gabs[:])
                            else:
                                nc.vector.tensor_tensor(gmax[:], gmax[:], gabs[:], Alu.max)
                        ftmp = stage.tile([64, nb], FP16, tag="foldt")
                        nc.vector.tensor_copy(ftmp[0:64], gmax[64:128])
                        nc.vector.tensor_tensor(gmax[0:64], gmax[0:64], ftmp[0:64], Alu.max)
                        nc.vector.tensor_copy(ftmp[0:32], gmax[32:64])
                        nc.vector.tensor_tensor(gmax[0:32], gmax[0:32], ftmp[0:32], Alu.max)
                        gmt = stage.tile([32, nb], FP16, tag="foldt")
                        nc.vector.transpose(gmt[:], gmax[0:32, :])
                        red = stage.tile([32, NT32], F32, tag="red")
                        nc.vector.tensor_reduce(
                            red[:], gmt[:].rearrange("p (c q) -> p c q", q=32),
                            axis=mybir.AxisListType.X, op=Alu.max,
                        )
                        if half == 0:
                            nc.vector.tensor_copy(hred_acc[blk][:], red[:])
                        else:
                            nc.vector.tensor_tensor(hred_acc[blk][:], hred_acc[blk][:],
                                                    red[:], Alu.max)

                for blk in range(NBLK):
                    nc.sync.dma_start(
                        hmax_in[p][blk * nb:(blk + 1) * nb].rearrange("(c p) -> p c", p=32),
                        hred_acc[blk][:],
                    )
                nc.gpsimd.collective_compute(
                    "AllReduce", Alu.max, replica_groups=rg,
                    ins=[hmax_in[p][:].opt()], outs=[hmax_out[_rep][:].opt()],
                )

                # ---- W2 |.| sums + AR + scales (overlaps fc1 compute) ----
                if rep_w2abs or _rep == 0:
                    with tc.tile_wait_until(W + 0.6):
                        acc2 = abs_sum_partial(w2t_e, IS, H, "w2")
                        wsum_store(acc2, wsum2_in[p], "w2")
                        nc.gpsimd.collective_compute(
                            "AllReduce", Alu.add, replica_groups=rg,
                            ins=[wsum2_in[p][:].opt()], outs=[wsum2_out[_rep][:].opt()],
                        )
                        rsw2_col, sw2_127_col = make_scales(wsum2_out[_rep], "w2")

                # ---------------- fc2 (inline W2 quantization) ----------------
                if KI >= 4:
                    splits = [(0, KI // 2, "bigw", bigw),
                              (KI // 2, (3 * KI) // 4, "xq", xqp),
                              ((3 * KI) // 4, KI, "xq", xqp)]
                else:
                    splits = [(0, KI, "bigw", bigw)]
                w2_tiles = []
                tc.tile_set_cur_wait(W + 2.4)
                for (k0, k1, tag, pool) in splits:
                    wt2 = pool.tile([128, k1 - k0, H], BF16, tag=tag)
                    for ki in range(k0, k1):
                        for c0 in range(0, H, CHUNK):
                            cw = min(CHUNK, H - c0)
                            wf = wqw.tile([128, CHUNK], F32, tag="wq32")
                            nc.scalar.dma_start(
                                wf[:, :cw],
                                w2t_e[ki * 128:(ki + 1) * 128, c0:c0 + cw],
                            )
                            quant_into(wt2[:, ki - k0, c0:c0 + cw], wf[:, :cw], cw, rsw2_col)
                    w2_tiles.append((k0, k1, wt2))

                def w2_lhsT(ki, msl):
                    for (k0, k1, wt2) in w2_tiles:
                        if k0 <= ki < k1:
                            return wt2[:, ki - k0, msl]
                    raise AssertionError

                for blk in range(NBLK):
                    tc.tile_set_cur_wait(W + 2.5)
                    s_row = stage.tile([1, nb], F32, tag="srow")
                    nc.sync.dma_start(
                        s_row[:],
                        hmax_out[_rep][blk * nb:(blk + 1) * nb].rearrange("(a f) -> a f", a=1),
                    )
                    nc.vector.tensor_scalar_max(s_row[:], s_row[:], EPS)
                    r_row = stage.tile([1, nb], F32, tag="srow")
                    nc.vector.reciprocal(r_row[:], s_row[:])
                    rq_t = bcast_row(r_row[:], nb)
                    nc.vector.tensor_scalar_mul(rq_t[:], rq_t[:], 127.0)

                    hq = hqp.tile([128, KI, nb], BF16, tag="hq")
                    for ki in range(KI):
                        ht = work.tile([128, nb], F32, tag="tmpf")
                        hin = work.tile([128, nb], FP16, tag="gq")
                        nc.sync.dma_start(
                            hin[:], h_dram[p][ki * 128:(ki + 1) * 128, blk * nb:(blk + 1) * nb]
                        )
                        nc.vector.tensor_tensor(ht[:], hin[:], rq_t[:], Alu.mult)
                        nc.vector.tensor_scalar(hq[:, ki, :], ht[:], MAGIC, MAGIC,
                                                Alu.add, Alu.subtract)
                    for m in range(MH):
                        ps = psum.tile([128, nb], F32, tag="ps1")
                        msl = slice(m * 128, (m + 1) * 128)
                        for ki in range(KI):
                            nc.tensor.matmul(
                                ps[:], lhsT=w2_lhsT(ki, msl), rhs=hq[:, ki, :],
                                start=(ki == 0), stop=(ki == KI - 1),
                            )
                        ot = outp.tile([128, nb], FP16, tag="ot")
                        nc.scalar.copy(ot[:], ps[:])
                        nc.sync.dma_start(
                            rs_in[p][blk // RSG][m * 128:(m + 1) * 128,
                                                 (blk % RSG) * nb:(blk % RSG + 1) * nb],
                            ot[:])
                    if blk % RSG == RSG - 1:
                        grp = blk // RSG
                        nc.gpsimd.collective_compute(
                            "ReduceScatter", Alu.add, replica_groups=rg,
                            ins=[rs_in[p][grp][:].opt()], outs=[rs_out[_rep][grp][:].opt()],
                        )
                        # post-RS per-token scaling on the owned H-slice
                        for bj in range(grp * RSG, (grp + 1) * RSG):
                            sj_row = stage.tile([1, nb], F32, tag="srow")
                            nc.sync.dma_start(
                                sj_row[:],
                                hmax_out[_rep][bj * nb:(bj + 1) * nb].rearrange(
                                    "(a f) -> a f", a=1),
                            )
                            nc.vector.tensor_scalar_max(sj_row[:], sj_row[:], EPS)
                            m2b_t = bcast_row(sj_row[:], nb)
                            nc.vector.tensor_scalar(m2b_t[:], m2b_t[:], sw2_127_col[:],
                                                    None, Alu.mult)
                            for q0 in range(0, HS, 128):
                                qw = min(128, HS - q0)
                                rt = outp.tile([128, nb], FP16, tag="rt")
                                nc.sync.dma_start(
                                    rt[:qw],
                                    rs_out[_rep][grp][q0:q0 + qw,
                                                      (bj % RSG) * nb:(bj % RSG + 1) * nb],
                                )
                                of = outp.tile([128, nb], F32, tag="of")
                                nc.vector.tensor_tensor(of[:qw], rt[:qw], m2b_t[:qw], Alu.mult)
                                nc.sync.dma_start(
                                    out_e[q0:q0 + qw, bj * nb:(bj + 1) * nb], of[:qw]
                                )

    nc.compile()
    return nc


_PROGRAM_CACHE = {}


def _get_program(key):
    if key not in _PROGRAM_CACHE:
        _PROGRAM_CACHE[key] = build_program(*key)
    return _PROGRAM_CACHE[key]


def make_in_maps(x, W1, W2, ncores=NCORES):
    t, h = x.reshape(-1, x.shape[-1]).shape
    i = W1.shape[0]
    xf = np.ascontiguousarray(x.reshape(t, h), dtype=np.float32)
    ts, isd = t // ncores, i // ncores
    in_maps = []
    for c in range(ncores):
        xs = xf[c * ts:(c + 1) * ts]
        in_maps.append({
            "xt": np.ascontiguousarray(xs.T),
            "w1t": np.ascontiguousarray(W1[c * isd:(c + 1) * isd, :].T, dtype=np.float32),
            "w2t": np.ascontiguousarray(W2[:, c * isd:(c + 1) * isd].T, dtype=np.float32),
        })
    return in_maps


def run(x, W1, W2, trace=False, trace_kwargs=None):
    """Run the distributed kernel on full inputs. Returns (out, BassKernelResults)."""
    t, h = x.reshape(-1, x.shape[-1]).shape
    i = W1.shape[0]
    nc = _get_program((t, h, i, NCORES))
    in_maps = make_in_maps(x, W1, W2)
    res = bass_utils.run_bass_kernel_spmd(
        nc, in_maps, core_ids=list(range(NCORES)), trace=trace,
        **(trace_kwargs or {}),
    )
    out_t = np.concatenate([res.results[c]["out_t"] for c in range(NCORES)], axis=0)
    out = np.ascontiguousarray(out_t.T).reshape(x.shape)
    return out, res


def kernel(x, W1, W2):
    out, _ = run(x, W1, W2)
    return out


class TimedRunner:
    """Compile once, keep inputs on device, time repeated executions.

    Mirrors bass2jax.run_bass_via_pjrt's multi-core path but persists the
    device-side inputs so repeat calls measure (dispatch + HW execution)
    only, not the host->device staging.
    """

    def __init__(self, nc, in_maps):
        import jax
        import concourse.mybir as mybir_
        from concourse import bass2jax
        from jax.experimental.shard_map import shard_map
        from jax.sharding import Mesh, PartitionSpec, NamedSharding

        bass2jax.install_neuronx_cc_hook()
        n_cores = len(in_maps)
        partition_name = nc.partition_id_tensor.name if nc.partition_id_tensor else None
        in_names, out_names, out_avals = [], [], []
        for alloc in nc.m.functions[0].allocations:
            if not isinstance(alloc, mybir_.MemoryLocationSet):
                continue
            name = alloc.memorylocations[0].name
            if alloc.kind == "ExternalInput":
                if name != partition_name:
                    in_names.append(name)
            elif alloc.kind == "ExternalOutput":
                out_names.append(name)
                out_avals.append(jax.core.ShapedArray(
                    tuple(alloc.tensor_shape), mybir_.dt.np(alloc.dtype)))
        n_params = len(in_names)
        n_outs = len(out_avals)
        all_in_names = list(in_names) + list(out_names)
        if partition_name is not None:
            all_in_names.append(partition_name)
        donate = tuple(range(n_params, n_params + n_outs))

        def _body(*args):
            operands = list(args)
            if partition_name is not None:
                operands.append(bass2jax.partition_id_tensor())
            outs = bass2jax._bass_exec_p.bind(
                *operands,
                out_avals=tuple(out_avals),
                in_names=tuple(all_in_names),
                out_names=tuple(out_names),
                lowering_input_output_aliases=(),
                sim_require_finite=True,
                sim_require_nnan=True,
                nc=nc,
            )
            return tuple(outs)

        devices = jax.devices()[:n_cores]
        mesh = Mesh(np.asarray(devices), ("core",))
        in_specs = (PartitionSpec("core"),) * (n_params + n_outs)
        out_specs = (PartitionSpec("core"),) * n_outs
        self._fn = jax.jit(
            shard_map(_body, mesh=mesh, in_specs=in_specs, out_specs=out_specs,
                      check_rep=False),
            donate_argnums=donate, keep_unused=True,
        )
        sh = NamedSharding(mesh, PartitionSpec("core"))
        concat_in = [
            np.concatenate([np.asarray(in_maps[c][nm]) for c in range(n_cores)], axis=0)
            for nm in in_names
        ]
        self._dev_in = [jax.device_put(a, sh) for a in concat_in]
        self._zero_shapes = [(n_cores * a.shape[0], *a.shape[1:]) for a in out_avals]
        self._zero_dtypes = [a.dtype for a in out_avals]
        self._sh = sh
        self._jax = jax
        self.out_names = out_names
        self.out_avals = out_avals
        self.n_cores = n_cores

    def run_once(self):
        import time
        jax = self._jax
        zeros = [jax.device_put(np.zeros(s, d), self._sh)
                 for s, d in zip(self._zero_shapes, self._zero_dtypes)]
        jax.block_until_ready(zeros)
        t0 = time.perf_counter()
        outs = self._fn(*self._dev_in, *zeros)
        jax.block_until_ready(outs)
        dt = time.perf_counter() - t0
        results = [
            {nm: np.asarray(outs[i]).reshape(self.n_cores, *self.out_avals[i].shape)[c]
             for i, nm in enumerate(self.out_names)}
            for c in range(self.n_cores)
        ]
        return results, dt

    def run_batch(self, k):
        """Launch k executions back-to-back; block once at the end.

        Returns (total_seconds, results_of_last_iteration). Each iteration
        consumes its own donated output buffers; inputs are shared
        (read-only). Per-device execution order serializes iterations, and
        collectives rendezvous ranks within each iteration, so iterations
        are race-free. total/k amortizes dispatch overhead and measures
        steady-state per-execution time.
        """
        import time
        jax = self._jax
        zsets = [
            [jax.device_put(np.zeros(s, d), self._sh)
             for s, d in zip(self._zero_shapes, self._zero_dtypes)]
            for _ in range(k)
        ]
        for zs in zsets:
            jax.block_until_ready(zs)
        t0 = time.perf_counter()
        outs = None
        for zs in zsets:
            outs = self._fn(*self._dev_in, *zs)
        jax.block_until_ready(outs)
        dt = time.perf_counter() - t0
        results = [
            {nm: np.asarray(outs[i]).reshape(self.n_cores, *self.out_avals[i].shape)[c]
             for i, nm in enumerate(self.out_names)}
            for c in range(self.n_cores)
        ]
        return dt, results

